# revision 26
# baseline (speedup 1.0000x reference)
"""Trainium2 Bass kernel for AttentionWithFP4Projections.

Sharding: tensor-parallel over heads across 8 cores (4 heads each, both
batches). Each core computes q/k/v for its 256 output dims, full causal
attention for its heads, and a partial o_proj (its 256-dim slice of the
contraction); partials are summed on the host (no device reduce).

v5 (over v3):
 - x is fp4-fake-quantized AND transposed on the host, then replicated
   to every core as a plain input: no device-side x quant pipeline, no
   x transposes, and no AllGather (v3's PE-starving serial collective)
 - host data layouts give 16-32KB contiguous DMA lines (weights and x
   arrive as straight 2D copies); x split across both HWDGE queues
 - o_proj deferred one qc behind attention so the o-quant vector chain
   and the oq transposes never stall the in-order PE queue
 - Wo resident in SBUF (loaded once); o_proj in fp16 (was bf16)
 - all matmul operands fp16 (FWL hides weight loads; measured
   rel_err 1.30e-2 vs 2e-2 gate); output partials fp16, f32-summed on
   host; all transposes fp16 single-pass on the PE
 - oT evacuation copy split scalar/vector halves; o-quant on 512-wide
   segments; merged v transposes; causal masks on vector
"""
import sys
import types
from contextlib import ExitStack

import numpy as np

# The NTFF profiling hook module is missing in this image; shim it so
# run_bass_kernel_spmd(trace=True) works (used by test.py, harmless here).
if 'antenv.axon_hooks' not in sys.modules:
    _m = types.ModuleType('antenv.axon_hooks')
    _m._hook = None
    _m.set_axon_ntff_profile_hook = lambda h: setattr(_m, '_hook', h)
    _m.get_axon_ntff_profile_hook = lambda: _m._hook
    sys.modules['antenv.axon_hooks'] = _m
    try:
        from trn_agent_boot.trn_boot import _ntff_profile_via_ctypes
        _m._hook = _ntff_profile_via_ctypes('/opt/axon/libaxon_pjrt.so')
    except Exception:
        pass

import concourse.mybir as mybir
import concourse.tile as tile
from concourse import bacc
from concourse import bass_utils
from concourse.masks import make_identity

F32 = mybir.dt.float32
F32R = mybir.dt.float32r
FP16 = mybir.dt.float16
BF16 = mybir.dt.bfloat16
I32 = mybir.dt.int32
ALU = mybir.AluOpType
ACTF = mybir.ActivationFunctionType

MMDT = FP16                   # matmul operand dtype for qkv/attention
NCORES = 8
B, S, HID = 2, 2048, 2048
T = B * S                     # 4096 tokens
NH, HD = 32, 64               # heads, head dim
HPC = NH // NCORES            # 4 heads per core
OD = HPC * HD                 # 256 output dims per core
NCH = S // 512                # 512-token chunks per batch
QW = 512                      # quant working width
MAGIC = 6291456.0             # 1.5*2^22: +/- rounds fp32 to multiples of 0.5


def _quant(nc, sb_tmp, out_ap, in_ap, scale_ap, rs6_ap, P=128):
    """FP4 fake-quant of in_ap [P, QW] -> out_ap, given per-16-block scale
    and rs6 (=6/amax) [P, QW//16].

    y   = x * rs6
    low = magic-round of y to multiples of 0.5   (covers |y| <= 2)
    hi  = (y_bits + 0x1FFFFF) & ~0x3FFFFF        (round-half-down to one
           mantissa bit; covers 2 < |y| <= 6; sign bit untouched)
    q   = |y| > 2 ? hi : low;  out = q * scale
    """
    nb = QW // 16
    y = sb_tmp.tile([128, QW], F32, tag="qt_y", name="qt_y")[:P, :]
    nc.vector.tensor_tensor(
        out=y.rearrange("p (b s) -> p b s", s=16),
        in0=in_ap.rearrange("p (b s) -> p b s", s=16),
        in1=rs6_ap.unsqueeze(2).broadcast_to([P, nb, 16]),
        op=ALU.mult)
    low = sb_tmp.tile([128, QW], F32, tag="qt_l", name="qt_l")[:P, :]
    nc.vector.tensor_scalar(out=low, in0=y,
                            scalar1=MAGIC, scalar2=MAGIC,
                            op0=ALU.add, op1=ALU.subtract)
    hi = sb_tmp.tile([128, QW], I32, tag="qt_h", name="qt_h")[:P, :]
    nc.vector.tensor_scalar(out=hi, in0=y.bitcast(I32),
                            scalar1=0x1FFFFF, scalar2=None, op0=ALU.add)
    nc.vector.tensor_scalar(out=hi, in0=hi,
                            scalar1=-4194304, scalar2=None,
                            op0=ALU.bitwise_and)
    # |y| then float-compare vs 2.0, in place in y's buffer (y dead after)
    nc.vector.tensor_scalar(out=y.bitcast(I32), in0=y.bitcast(I32),
                            scalar1=0x7FFFFFFF, scalar2=None,
                            op0=ALU.bitwise_and)
    pred = y.bitcast(I32)
    nc.vector.tensor_scalar(out=pred, in0=y,
                            scalar1=2.0, scalar2=None, op0=ALU.is_gt)
    nc.vector.copy_predicated(low, pred, hi.bitcast(F32))
    nc.vector.tensor_tensor(
        out=out_ap.rearrange("p (b s) -> p b s", s=16),
        in0=low.rearrange("p (b s) -> p b s", s=16),
        in1=scale_ap.unsqueeze(2).broadcast_to([P, nb, 16]),
        op=ALU.mult)


def _amax_scales(nc, sb_tmp, in_ap, P=128):
    """Returns (rs6, amax) [P, QW//16] tiles for fp4 quant of
    in_ap [P, QW]. amax is clamped in place; rs6 shares rcp's buffer."""
    nb = QW // 16
    amax = sb_tmp.tile([128, 32], F32, tag="am", name="am")[:P, :nb]
    nc.vector.tensor_reduce(amax, in_ap.rearrange("p (b s) -> p b s", s=16),
                            axis=mybir.AxisListType.X, op=ALU.max,
                            apply_absolute_value=True)
    nc.vector.tensor_scalar_max(amax, amax, 1e-30)
    rcp = sb_tmp.tile([128, 32], F32, tag="rc", name="rc")[:P, :nb]
    nc.vector.reciprocal(rcp, amax)
    rs6 = rcp
    nc.vector.tensor_scalar_mul(rs6, rcp, 6.0)
    return rs6, amax


def build():
    nc = bacc.Bacc("TRN2", target_bir_lowering=False, debug=False,
                   num_devices=NCORES)
    # x: per 512-token chunk, [128 part, 16 ktiles * 512 tok] contiguous
    x_d = nc.dram_tensor("xC", [B * NCH, 128, 16 * 512], MMDT,
                         kind="ExternalInput").ap()
    wq_d = nc.dram_tensor("wqT", [128, 16 * OD], MMDT,
                          kind="ExternalInput").ap()
    wk_d = nc.dram_tensor("wkT", [128, 16 * OD], MMDT,
                          kind="ExternalInput").ap()
    wv_d = nc.dram_tensor("wvT", [128, 16 * OD], MMDT,
                          kind="ExternalInput").ap()
    wo_d = nc.dram_tensor("woT", [128, 2 * HID], FP16,
                          kind="ExternalInput").ap()
    cos_d = nc.dram_tensor("cosT", [128, S], MMDT, kind="ExternalInput").ap()
    sin_d = nc.dram_tensor("sinTs", [128, S], MMDT,
                           kind="ExternalInput").ap()
    mask_d = nc.dram_tensor("masks", [128, 128], MMDT,
                            kind="ExternalInput").ap()
    out_d = nc.dram_tensor("partialT", [HID, T], FP16,
                           kind="ExternalOutput").ap()

    with tile.TileContext(nc) as tc, ExitStack() as ctx:
        sb_w = ctx.enter_context(tc.tile_pool(name="sb_w", bufs=1))
        sb_tmp = ctx.enter_context(tc.tile_pool(name="sb_tmp", bufs=1))
        sb_io = ctx.enter_context(tc.tile_pool(name="sb_io", bufs=2))
        sb_att = ctx.enter_context(tc.tile_pool(name="sb_att", bufs=1))
        sb_x = ctx.enter_context(tc.tile_pool(name="sb_x", bufs=6))
        sb_pt = ctx.enter_context(tc.tile_pool(name="sb_pt", bufs=2))
        sb_po = ctx.enter_context(tc.tile_pool(name="sb_po", bufs=3))
        ps_sc = ctx.enter_context(
            tc.tile_pool(name="ps_sc", bufs=2, space="PSUM"))
        ps_ot = ctx.enter_context(
            tc.tile_pool(name="ps_ot", bufs=1, space="PSUM"))
        ps_mm = ctx.enter_context(
            tc.tile_pool(name="ps_mm", bufs=2, space="PSUM"))

        ident = sb_w.tile([128, 128], F32)
        make_identity(nc, ident[:])
        ident_h = sb_w.tile([128, 128], FP16)
        nc.vector.tensor_copy(ident_h[:], ident[:])
        masks = sb_w.tile([128, 128], MMDT)
        nc.sync.dma_start(masks[:], mask_d)

        # x stream: all chunks on the sync queue, issued ahead of use
        _xq = []
        _xdone = set()

        def issue_x(b_, ch_, split=1):
            if (b_, ch_) in _xdone:
                return
            _xdone.add((b_, ch_))
            for h2 in range(2):
                xt = sb_x.tile([128, 8 * 512], MMDT, tag="xh", name="xh")
                w_ = 4096 // split
                for s_ in range(split):
                    nc.sync.dma_start(
                        xt[:, s_ * w_:(s_ + 1) * w_],
                        x_d[b_ * NCH + ch_]
                        [:, h2 * 4096 + s_ * w_:h2 * 4096 + (s_ + 1) * w_])
                _xq.append(xt)

        issue_x(0, 0, split=4)
        # weights: pre-quantized, transposed, SBUF-layout on host.
        # wk first (first matmuls need it), halves split across queues.
        wT = {}
        for nm, wd in (("k", wk_d), ("q", wq_d), ("v", wv_d)):
            wt = sb_w.tile([128, 16 * OD], MMDT, name=f"w{nm}T")
            wT[nm] = wt
            half = 8 * OD
            if nm == "k":
                nc.scalar.dma_start(wt[:], wd)
            else:
                nc.scalar.dma_start(wt[:, 0:half], wd[:, 0:half])
                nc.sync.dma_start(wt[:, half:2 * half],
                                  wd[:, half:2 * half])
        wo_sb = sb_w.tile([128, 2 * HID], FP16, name="wo_sb")
        nc.scalar.dma_start(wo_sb[:], wo_d)

        # persistent per-batch buffers
        qT = [sb_att.tile([128, S], MMDT, name=f"qT{m}") for m in range(2)]
        kT = [sb_att.tile([128, S], MMDT, name=f"kT{m}") for m in range(2)]
        vE = [sb_att.tile([128, 16 * 65], MMDT, name=f"vE{h}")
              for h in range(HPC)]
        oqT = [sb_att.tile([128, S], FP16, name=f"oqT{m}") for m in range(2)]
        onat = sb_att.tile([128, 1024], F32, name="onat")
        rsum = sb_tmp.tile([128, 16], F32, name="rsum")
        rraw = sb_tmp.tile([128, 16], F32, name="rraw")

        # ones columns of vE: written once, never overwritten after
        ones = sb_tmp.tile([128, 16], F32, name="ones")
        nc.vector.memset(ones[:], 1.0)
        for h in range(HPC):
            nc.vector.tensor_copy(
                vE[h][:].rearrange("p (k c) -> p k c", c=65)[:, :, 64:65],
                ones[:].unsqueeze(2))

        def rope_piece(dst, m, ch, cosT, sinT):
            """RoPE in place on dst[m][:, ch*512:(ch+1)*512]."""
            c0 = ch * 512
            sh = sb_io.tile([128, 512], MMDT, tag="rope_sh", bufs=1)
            for hh in range(2):
                p0 = hh * 64
                nc.gpsimd.dma_start(
                    sh[p0:p0 + 32, :],
                    dst[m][p0 + 32:p0 + 64, c0:c0 + 512])
                nc.gpsimd.dma_start(
                    sh[p0 + 32:p0 + 64, :],
                    dst[m][p0:p0 + 32, c0:c0 + 512])
            nc.vector.tensor_tensor(out=sh[:], in0=sh[:],
                                     in1=sinT[:], op=ALU.mult)
            # in place: the cos multiply write waits on the shuffle reads
            nc.vector.tensor_tensor(
                out=dst[m][:, c0:c0 + 512], in0=dst[m][:, c0:c0 + 512],
                in1=cosT[:], op=ALU.mult)
            nc.vector.tensor_tensor(
                out=dst[m][:, c0:c0 + 512], in0=dst[m][:, c0:c0 + 512],
                in1=sh[:], op=ALU.add)

        def emit_oproj(b, qc, oqs):
            """Transpose qc's quantized o into oqT, then o_proj + store.
            Deferred one qc behind attention so the quant chain (vector)
            never stalls the in-order PE queue."""
            t0 = b * S
            for q4 in range(2):
                for tt2 in range(2):
                    for j in range(2):
                        ptq = ps_mm.tile([128, 512], FP16, tag="ps_mm",
                                         name="ptq")[:, 0:128]
                        nc.tensor.transpose(
                            ptq,
                            oqs[q4][:, tt2 * 256 + j * 128:
                                    tt2 * 256 + (j + 1) * 128],
                            ident_h[:])
                        nc.vector.tensor_copy(
                            oqT[j][:, qc * 512 + (q4 * 2 + tt2) * 128:
                                   qc * 512 + (q4 * 2 + tt2 + 1) * 128],
                            ptq)
            for mo4 in range(4):
                posb = sb_po.tile([128, 4 * 512], FP16, tag="posb",
                                  name="posb")
                for mi in range(4):
                    mo = mo4 * 4 + mi
                    po = ps_mm.tile([128, 512], F32, tag="ps_mm",
                                    name="po")
                    for i in range(2):
                        nc.tensor.matmul(
                            po[:],
                            wo_sb[:, i * HID + mo * 128:
                                  i * HID + (mo + 1) * 128],
                            oqT[i][:, qc * 512:(qc + 1) * 512],
                            start=(i == 0), stop=(i == 1))
                    if mi == 0:
                        nc.scalar.copy(
                            posb[:, mi * 512:(mi + 1) * 512], po[:])
                    else:
                        nc.vector.tensor_copy(
                            posb[:, mi * 512:(mi + 1) * 512], po[:])
                nc.gpsimd.dma_start(
                    out_d.rearrange("(a p) t -> p a t", p=128)
                    [:, mo4 * 4:(mo4 + 1) * 4,
                     t0 + qc * 512:t0 + (qc + 1) * 512],
                    posb[:].rearrange("p (a t) -> p a t", a=4))

        for b in range(B):
            t0 = b * S

            # -------- projections over 512-token chunks --------
            for ch in range(NCH):
                cc0 = ch * 512
                cosT = sb_io.tile([128, 512], MMDT, tag="rope_c", bufs=2)
                sinT = sb_io.tile([128, 512], MMDT, tag="rope_s", bufs=2)
                nc.scalar.dma_start(cosT[:], cos_d[:, cc0:cc0 + 512])
                nc.scalar.dma_start(sinT[:], sin_d[:, cc0:cc0 + 512])
                xh = [_xq.pop(0), _xq.pop(0)]
                for nm in ("k", "q", "v"):
                    for m in range(2):
                        pj = ps_mm.tile([128, 512], F32, tag="ps_mm",
                                        name="pj")
                        for i in range(16):
                            nc.tensor.matmul(
                                pj[:],
                                wT[nm][:, i * OD + m * 128:
                                       i * OD + (m + 1) * 128],
                                xh[i // 8][:, (i % 8) * 512:
                                           (i % 8 + 1) * 512],
                                start=(i == 0), stop=(i == 15))
                        if nm == "v":
                            vsb = sb_io.tile([128, 512], FP16, tag="vsb",
                                             bufs=3)
                            nc.vector.tensor_copy(vsb[:], pj[:])
                            for kt in range(4):
                                ptv = ps_mm.tile([128, 512], FP16,
                                                 tag="ps_mm",
                                                 name="ptv")[:, 0:128]
                                nc.tensor.transpose(
                                    ptv, vsb[:, kt * 128:(kt + 1) * 128],
                                    ident_h[:])
                                ktile = ch * 4 + kt
                                for hh in range(2):
                                    nc.vector.tensor_copy(
                                        vE[m * 2 + hh][:, ktile * 65:
                                                       ktile * 65 + 64],
                                        ptv[:, hh * 64:hh * 64 + 64])
                        else:
                            dst = qT if nm == "q" else kT
                            if nm == "q":
                                nc.scalar.copy(dst[m][:, cc0:cc0 + 512],
                                               pj[:])
                            else:
                                nc.vector.tensor_copy(
                                    dst[m][:, cc0:cc0 + 512], pj[:])
                            rope_piece(dst, m, ch, cosT, sinT)
                if ch + 1 < NCH:
                    issue_x(b, ch + 1)
                elif b + 1 < B:
                    issue_x(b + 1, 0)

            # -------- attention; o_proj deferred one qc behind --------
            if b + 1 < B:
                issue_x(b + 1, 1)
            pending = None
            for qc in range(4):
                for m in range(2):
                    oT = ps_ot.tile([65, 1024], F32, tag="ps_oT",
                                    name="ps_oT")
                    for kblk in range(4 * qc + 4):
                        qs = max(qc * 512, kblk * 128)
                        w = (qc + 1) * 512 - qs
                        off = qs - qc * 512
                        diag = kblk >= 4 * qc
                        sc2 = ps_sc.tile([128, 1024], F32, tag="sc2",
                                         name="sc2")
                        for hh in range(2):
                            p0 = hh * 64
                            nc.tensor.matmul(
                                sc2[:, hh * 512: hh * 512 + w],
                                kT[m][p0:p0 + 64,
                                      kblk * 128:(kblk + 1) * 128],
                                qT[m][p0:p0 + 64, qs:(qc + 1) * 512],
                                start=True, stop=True)
                        pT = sb_pt.tile([128, 1024], MMDT, tag="pT",
                                        name="pT")
                        if w == 512:
                            nc.scalar.activation(pT[:], sc2[:],
                                                 ACTF.Exp, scale=0.125)
                        else:
                            nc.scalar.activation(
                                pT[:].rearrange("p (h w) -> p h w", h=2)
                                [:, :, 0:w],
                                sc2[:].rearrange("p (h w) -> p h w", h=2)
                                [:, :, 0:w],
                                ACTF.Exp, scale=0.125)
                        if diag:
                            for hh in range(2):
                                nc.vector.tensor_tensor(
                                    out=pT[:, hh * 512: hh * 512 + 128],
                                    in0=pT[:, hh * 512: hh * 512 + 128],
                                    in1=masks[:], op=ALU.mult)
                        for hh in range(2):
                            h_ = m * 2 + hh
                            nc.tensor.matmul(
                                oT[:, hh * 512 + off: hh * 512 + off + w],
                                vE[h_][:, kblk * 65:(kblk + 1) * 65],
                                pT[:, hh * 512: hh * 512 + w],
                                start=(kblk == 0),
                                stop=(kblk == 4 * qc + 3),
                                skip_group_check=(kblk == 4 * qc + 3
                                                  and off != 0))
                    # evacuate oT: copy split across engines, transpose
                    # to natural + collect row sums (fp16 single-pass)
                    osb = sb_pt.tile([128, 1024], FP16, tag="pT",
                                     name="osb", bufs=4)
                    nc.scalar.copy(osb[0:65, 0:512], oT[:, 0:512])
                    nc.vector.tensor_copy(osb[0:65, 512:1024],
                                          oT[:, 512:1024])
                    for hh in range(2):
                        h_ = m * 2 + hh
                        ptn = ps_mm.tile([128, 512], FP16, tag="ps_mm",
                                         name="ptn")[:, 0:264]
                        for tt in range(4):
                            nc.tensor.transpose(
                                ptn[:, tt * 66:tt * 66 + 65],
                                osb[0:65, hh * 512 + tt * 128:
                                    hh * 512 + (tt + 1) * 128],
                                ident_h[0:65, 0:65])
                        nc.vector.tensor_copy(
                            onat[:].rearrange("p (tt g) -> p tt g", g=256)
                            [:, :, h_ * 64:(h_ + 1) * 64],
                            ptn.rearrange("p (tt c) -> p tt c", c=66)
                            [:, :, 0:64])
                        nc.vector.tensor_copy(
                            rraw[:].rearrange("p (tt g) -> p tt g", g=4)
                            [:, :, h_:h_ + 1],
                            ptn.rearrange("p (tt c) -> p tt c", c=66)
                            [:, :, 64:65])
                nc.vector.reciprocal(rsum[:], rraw[:])
                # quantize [128, 512] halves (two tt groups), fold 1/sum
                oqs = []
                for q4 in range(2):
                    seg = onat[:, q4 * 512:(q4 + 1) * 512]
                    rs6, amax = _amax_scales(nc, sb_tmp, seg)
                    sct = sb_tmp.tile([128, 32], F32, tag="sc", name="sct")
                    nc.vector.tensor_tensor(
                        out=sct[:].rearrange("p (a h s) -> p a h s",
                                             a=2, s=4),
                        in0=amax.rearrange("p (a h s) -> p a h s",
                                           a=2, s=4),
                        in1=rsum[:, q4 * 8:(q4 + 1) * 8]
                        .rearrange("p (a h) -> p a h", a=2)
                        .unsqueeze(3).broadcast_to([128, 2, 4, 4]),
                        op=ALU.mult)
                    nc.vector.tensor_scalar_mul(sct[:], sct[:], 1.0 / 6.0)
                    oq = sb_tmp.tile([128, QW], FP16, tag="oq", name="oq",
                                     bufs=4)
                    _quant(nc, sb_tmp, oq[:], seg, sct[:], rs6)
                    oqs.append(oq)
                if pending is not None:
                    emit_oproj(b, pending[0], pending[1])
                pending = (qc, oqs)
            emit_oproj(b, pending[0], pending[1])

    nc.compile()
    return nc


def _np_quant(x):
    """Host fp4 fake-quant, matching the reference implementation."""
    sh = x.shape
    xb = x.reshape(sh[:-1] + (sh[-1] // 16, 16)).astype(np.float32)
    amax = np.max(np.abs(xb), axis=-1, keepdims=True).astype(np.float32)
    amax_c = np.maximum(amax, np.float32(1e-30))
    rcp = (np.float32(1.0) / amax_c).astype(np.float32)
    rs6 = (rcp * np.float32(6.0)).astype(np.float32)
    scale = (amax * np.float32(1.0 / 6.0)).astype(np.float32)
    y = (xb * rs6).astype(np.float32)
    yi = y.view(np.int32)
    hi = ((yi + np.int32(0x1FFFFF)) & np.int32(-4194304)).view(np.float32)
    M32 = np.float32(MAGIC)
    low = ((y + M32).astype(np.float32) - M32).astype(np.float32)
    q = np.where(np.abs(y) > np.float32(2.0), hi, low)
    return (q * scale).astype(np.float32).reshape(sh)


_HOST_CACHE = {}


def _host_tables():
    if _HOST_CACHE:
        return _HOST_CACHE
    D = HD
    inv = (1.0 / (10000.0 ** (np.arange(0, D, 2, dtype=np.float32)
                              / np.float32(D)))).astype(np.float32)
    fr = (np.arange(S, dtype=np.float32)[:, None] * inv[None, :]).astype(
        np.float32)
    cos = np.concatenate([np.cos(fr), np.cos(fr)], -1).astype(np.float32)
    sin = np.concatenate([np.sin(fr), np.sin(fr)], -1).astype(np.float32)
    sgn = np.where(np.arange(D) < D // 2, np.float32(-1.0), np.float32(1.0))
    cosT = np.tile(cos.T, (2, 1)).astype(np.float32)          # [128, S]
    sinTs = np.tile((sin * sgn[None, :]).T, (2, 1)).astype(np.float32)
    # 0/1 triangle for post-exp masking, sT layout: k-row kk allows q >= kk
    masks = np.zeros((128, 128), np.float32)
    for kk in range(128):
        masks[kk, kk:] = 1.0
    _HOST_CACHE.update(cosT=cosT, sinTs=sinTs, masks=masks)
    return _HOST_CACHE


_NC_CACHE = []


def make_in_maps(hidden_states, Wq, Wk, Wv, Wo):
    mmdt = mybir.dt.np(MMDT)
    fp16 = mybir.dt.np(FP16)
    tabs = _host_tables()
    xq = _np_quant(np.asarray(hidden_states, np.float32).reshape(T, HID))
    # chunk-major SBUF layout: [chunk, 128 part, 16 ktiles * 512 tok]
    xC = np.ascontiguousarray(
        xq.T.reshape(16, 128, B * NCH, 512).transpose(2, 1, 0, 3)
        .reshape(B * NCH, 128, 16 * 512)).astype(mmdt)
    wq_q = _np_quant(np.asarray(Wq, np.float32))
    wk_q = _np_quant(np.asarray(Wk, np.float32))
    wv_q = _np_quant(np.asarray(Wv, np.float32))
    wo_q = _np_quant(np.asarray(Wo, np.float32))
    cosT = tabs['cosT'].astype(mmdt)
    sinTs = tabs['sinTs'].astype(mmdt)
    masks = tabs['masks'].astype(mmdt)

    def wlay(w, sl):
        # [128 part, 16 ktiles * OD] SBUF layout, contiguous rows
        return np.ascontiguousarray(
            w[sl, :].T.reshape(16, 128, OD).transpose(1, 0, 2)
            .reshape(128, 16 * OD)).astype(mmdt)

    in_maps = []
    for c in range(NCORES):
        sl = slice(c * OD, (c + 1) * OD)
        woTc = np.ascontiguousarray(
            wo_q[:, sl].T.reshape(2, 128, HID).transpose(1, 0, 2)
            .reshape(128, 2 * HID)).astype(fp16)
        in_maps.append(dict(
            xC=xC, wqT=wlay(wq_q, sl), wkT=wlay(wk_q, sl),
            wvT=wlay(wv_q, sl), woT=woTc,
            cosT=cosT, sinTs=sinTs, masks=masks,
        ))
    return in_maps


def kernel(hidden_states, Wq, Wk, Wv, Wo):
    in_maps = make_in_maps(hidden_states, Wq, Wk, Wv, Wo)
    if not _NC_CACHE:
        _NC_CACHE.append(build())
    nc = _NC_CACHE[0]
    res = bass_utils.run_bass_kernel_spmd(nc, in_maps,
                                          core_ids=list(range(NCORES)))
    total = np.zeros((HID, T), np.float32)
    for r in res.results:
        total += np.asarray(r["partialT"], np.float32)
    return np.ascontiguousarray(total.T.reshape(B, S, HID))


if __name__ == "__main__":
    d = np.load('/root/problem/inputs.npz')
    out = kernel(d['hidden_states'], d['Wq'], d['Wk'], d['Wv'], d['Wo'])
    ref = np.load('/root/problem/ref_out.npy')
    rel2 = np.linalg.norm(out - ref) / np.linalg.norm(ref)
    print(f"relL2={rel2:.3e} absmax={np.abs(out - ref).max():.3e}")
